# revision 1
# baseline (speedup 1.0000x reference)
"""Fused cross-attention audio fuser (dense transformer block) on TRN2.

Strategy: pure batch data-parallelism across the 8 NeuronCores (B=16 -> 2
batches per core, no collectives). Per batch everything is kept
channel-major ([C, tokens]) so the only transposes needed are 8 PE block
transposes of the audio features:

  qT = Wq.T @ imgT (+bq, *scale)      [C_AUD, HW]     (imgT is native layout)
  kT = Wk.T @ audT (+bk)              [C_AUD, K_LEN]
  v  = audT.T @ Wv (+bv via K=1 mm)   [K_LEN, C_AUD]  (seq-major)
  S_hT = kT_h.T @ qT_h                [K_LEN, HW] per head (K=64, row-packed pairs)
  expS = exp(S_hT)                    (no max subtraction; scores are provably small)
  sumexp_hT = ones.T @ expS           (K=1-col matmul rows, col-tiled pairs)
  attnT_h = v_h.T @ expS              (col-tiled head pairs -> [128, HW] chunks)
  attnT normalized by 1/sumexp via gpsimd partition-broadcast + DVE mul
  projT = Wo.T @ attnT (+bo)          [C_IMG, HW]
  y = imgT + projT; layernorm over C via ones-matmul stats (sum, sum of squares)
"""

import numpy as np
from contextlib import ExitStack

import concourse.bass as bass
import concourse.mybir as mybir
import concourse.tile as tile
from concourse import bacc
from concourse.bass_utils import run_bass_kernel_spmd
from concourse.masks import make_identity

# Problem constants (hardcoded per spec)
B, C_IMG, H, W = 16, 512, 32, 32
C_AUD, K_LEN, N_HEADS = 512, 256, 8
HD = C_AUD // N_HEADS           # 64
HW = H * W                      # 1024
EPS = 1e-5
SCALE = float(HD) ** -0.5       # 0.125
N_CORES = 8
BPC = B // N_CORES              # 2 batches per core

F32 = mybir.dt.float32
BF16 = mybir.dt.bfloat16
P = 128
NCI = C_IMG // P                # 4 c_img chunks
NCA = C_AUD // P                # 4 c_aud chunks
NS = K_LEN // P                 # 2 seq chunks
NT = 512                        # matmul free-dim tile (one PSUM bank fp32)
NQ = HW // NT                   # 2 q tiles

# Matmul dtype mode: "f32" (exact, 4 cyc/row) or "f32r" (fast, reduced precision)
MM_MODE = "f32r"

Ident = mybir.ActivationFunctionType.Identity
Copy = mybir.ActivationFunctionType.Copy
Exp = mybir.ActivationFunctionType.Exp
Sqrt = mybir.ActivationFunctionType.Sqrt
ADD = mybir.AluOpType.add
SUB = mybir.AluOpType.subtract
MUL = mybir.AluOpType.mult


def _body(ctx: ExitStack, tc: tile.TileContext, mm_dt, dbg=False, repeat=1):
    nc = tc.nc

    MM = mm_dt                     # dtype for tiles feeding matmuls
    def ff(ap):
        # view an MM-typed AP as plain f32 for non-matmul consumers
        return ap.bitcast(F32) if mm_dt != F32 else ap

    img_d = nc.dram_tensor("img", [BPC, C_IMG, HW], MM, kind="ExternalInput").ap()
    aud_d = nc.dram_tensor("aud", [BPC, K_LEN, C_AUD], F32, kind="ExternalInput").ap()
    wq_d = nc.dram_tensor("wq", [C_IMG, C_AUD], MM, kind="ExternalInput").ap()
    wk_d = nc.dram_tensor("wk", [C_AUD, C_AUD], MM, kind="ExternalInput").ap()
    wv_d = nc.dram_tensor("wv", [C_AUD, C_AUD], MM, kind="ExternalInput").ap()
    wo_d = nc.dram_tensor("wo", [C_AUD, C_IMG], MM, kind="ExternalInput").ap()
    bq_d = nc.dram_tensor("bq", [C_AUD], F32, kind="ExternalInput").ap()
    bk_d = nc.dram_tensor("bk", [C_AUD], F32, kind="ExternalInput").ap()
    bv_d = nc.dram_tensor("bv", [C_AUD], MM, kind="ExternalInput").ap()
    bo_d = nc.dram_tensor("bo", [C_IMG], F32, kind="ExternalInput").ap()
    gam_d = nc.dram_tensor("gamma", [C_IMG], F32, kind="ExternalInput").ap()
    bet_d = nc.dram_tensor("beta", [C_IMG], F32, kind="ExternalInput").ap()
    out_d = nc.dram_tensor("out", [BPC, C_IMG, HW], F32, kind="ExternalOutput").ap()
    if dbg:
        dbg_d = {
            "qT": nc.dram_tensor("dbg_qT", [C_AUD, HW], F32, kind="ExternalOutput").ap(),
            "kT": nc.dram_tensor("dbg_kT", [C_AUD, K_LEN], F32, kind="ExternalOutput").ap(),
            "v": nc.dram_tensor("dbg_v", [K_LEN, C_AUD], F32, kind="ExternalOutput").ap(),
            "expS0": nc.dram_tensor("dbg_expS0", [K_LEN, HW], F32, kind="ExternalOutput").ap(),
            "expS1": nc.dram_tensor("dbg_expS1", [K_LEN, HW], F32, kind="ExternalOutput").ap(),
            "sumexp": nc.dram_tensor("dbg_sumexp", [P, HW], F32, kind="ExternalOutput").ap(),
            "attnT": nc.dram_tensor("dbg_attnT", [C_AUD, HW], F32, kind="ExternalOutput").ap(),
            "anum": nc.dram_tensor("dbg_anum", [P, HW], F32, kind="ExternalOutput").ap(),
            "rb": nc.dram_tensor("dbg_rb", [P, HW], F32, kind="ExternalOutput").ap(),
            "y": nc.dram_tensor("dbg_y", [C_IMG, HW], F32, kind="ExternalOutput").ap(),
        }
        dbg_pool = ctx.enter_context(tc.tile_pool(name="dbgp", bufs=2))

        def dump_f32(dst, src_ap, rows=P):
            t = dbg_pool.tile([rows, src_ap.shape[-1]], F32, tag="dbg")
            nc.scalar.activation(t[:], src_ap, Copy)
            nc.sync.dma_start(out=dst, in_=t[:])

    cpool = ctx.enter_context(tc.tile_pool(name="consts", bufs=1))
    wpool = ctx.enter_context(tc.tile_pool(name="weights", bufs=1))
    img_pool = ctx.enter_context(tc.tile_pool(name="img", bufs=2))
    aud_pool = ctx.enter_context(tc.tile_pool(name="aud", bufs=2))
    big_pool = ctx.enter_context(tc.tile_pool(name="big", bufs=1))       # qT, attnT, y
    kv_pool = ctx.enter_context(tc.tile_pool(name="kv", bufs=1))         # kT, audT, v
    expS_pool = ctx.enter_context(tc.tile_pool(name="expS", bufs=6))
    rb_pool = ctx.enter_context(tc.tile_pool(name="rbcast", bufs=3))
    row1_pool = ctx.enter_context(tc.tile_pool(name="row1", bufs=6))
    chunk_pool = ctx.enter_context(tc.tile_pool(name="chunk", bufs=4))   # proj/ysq/tmp/out
    mm_ps = ctx.enter_context(tc.tile_pool(name="mm_ps", bufs=6, space="PSUM"))
    fr_ps = ctx.enter_context(tc.tile_pool(name="fr_ps", bufs=2, space="PSUM"))

    # ---- constants / weights (loaded once) ----
    # memset doesn't codegen for f32r; stage in f32 then ACT-copy (which rounds)
    ones_f32 = cpool.tile([P, P + 1], F32, tag="ones_f32")
    nc.vector.memset(ones_f32[:], 1.0)
    ones_col = cpool.tile([P, 1], MM)
    nc.scalar.activation(ones_col[:], ones_f32[:, 0:1], Copy)
    ones_row = cpool.tile([1, P], MM)
    nc.scalar.activation(ones_row[:], ones_f32[0:1, 1:P + 1], Copy)
    ones_bf64 = cpool.tile([P, HD], BF16, tag="ones_bf")
    nc.vector.memset(ones_bf64[:], 1.0)
    ident = cpool.tile([P, P], F32)
    make_identity(nc, ident[:])
    ident_r = cpool.tile([P, P], MM, tag="ident_r")
    nc.scalar.activation(ident_r[:], ident[:], Copy)
    eps_col = cpool.tile([1, 1], F32, tag="eps")
    nc.vector.memset(eps_col[:], EPS)

    wq_sb = wpool.tile([P, NCI, C_AUD], MM, tag="wq")
    wk_sb = wpool.tile([P, NCA, C_AUD], MM, tag="wk")
    wv_sb = wpool.tile([P, NCA, C_AUD], MM, tag="wv")
    wo_sb = wpool.tile([P, NCA, C_IMG], MM, tag="wo")
    bq_col = cpool.tile([P, NCA], F32, tag="bq")
    bk_col = cpool.tile([P, NCA], F32, tag="bk")
    bo_col = cpool.tile([P, NCI], F32, tag="bo")
    gam_col = cpool.tile([P, NCI], F32, tag="gam")
    bet_col = cpool.tile([P, NCI], F32, tag="bet")
    bv_row = cpool.tile([1, C_AUD], MM, tag="bv")

    for rep in range(repeat):
        # input tiles for both batches; DMA emission order matters: feed the
        # first dependency chain (audio -> transposes, Wq+img -> qT) first.
        aud_tiles, img_tiles = [], []
        for b in range(BPC):
            aud_tiles.append(aud_pool.tile([P, NS, C_AUD], F32, tag="aud",
                                           name=f"aud_sb{b}"))
            img_tiles.append(img_pool.tile([P, NCI, HW], MM, tag="img",
                                           name=f"img_sb{b}"))
        for st in range(NS):
            nc.sync.dma_start(out=aud_tiles[0][:, st, :], in_=aud_d[0, st * P:(st + 1) * P, :])
        for ci in range(NCI):
            nc.sync.dma_start(out=wq_sb[:, ci, :], in_=wq_d[ci * P:(ci + 1) * P, :])
        for m in range(NCA):
            nc.sync.dma_start(out=bq_col[:, m:m + 1], in_=bq_d[m * P:(m + 1) * P])
            nc.sync.dma_start(out=bk_col[:, m:m + 1], in_=bk_d[m * P:(m + 1) * P])
        # fold the attention scale into q's bias: q_scaled = psum*SCALE + bq*SCALE
        nc.vector.tensor_scalar_mul(bq_col[:], bq_col[:], SCALE)
        for ci in range(NCI):
            nc.sync.dma_start(out=img_tiles[0][:, ci, :], in_=img_d[0, ci * P:(ci + 1) * P, :])
        for ci in range(NCA):
            nc.sync.dma_start(out=wk_sb[:, ci, :], in_=wk_d[ci * P:(ci + 1) * P, :])
            nc.sync.dma_start(out=wv_sb[:, ci, :], in_=wv_d[ci * P:(ci + 1) * P, :])
        nc.sync.dma_start(out=bv_row[:], in_=bv_d[:])
        for ci in range(NCA):
            nc.sync.dma_start(out=wo_sb[:, ci, :], in_=wo_d[ci * P:(ci + 1) * P, :])
        for m in range(NCI):
            nc.sync.dma_start(out=bo_col[:, m:m + 1], in_=bo_d[m * P:(m + 1) * P])
            nc.sync.dma_start(out=gam_col[:, m:m + 1], in_=gam_d[m * P:(m + 1) * P])
            nc.sync.dma_start(out=bet_col[:, m:m + 1], in_=bet_d[m * P:(m + 1) * P])
        for b in range(1, BPC):
            for st in range(NS):
                nc.sync.dma_start(out=aud_tiles[b][:, st, :], in_=aud_d[b, st * P:(st + 1) * P, :])
            for ci in range(NCI):
                nc.sync.dma_start(out=img_tiles[b][:, ci, :], in_=img_d[b, ci * P:(ci + 1) * P, :])

        for b in range(BPC):
            img_sb = img_tiles[b]
            aud_sb = aud_tiles[b]

            # ---- audT: transpose audio [s, c] -> [c, s] via PE ----
            audT_sb = kv_pool.tile([P, NCA, K_LEN], MM, tag="audT")
            for ci in range(NCA):
                tp = fr_ps.tile([P, K_LEN], F32, tag="fr")
                for st in range(NS):
                    nc.tensor.transpose(
                        tp[:, st * P:(st + 1) * P],
                        aud_sb[:, st, ci * P:(ci + 1) * P],
                        ident[:],
                    )
                nc.vector.tensor_copy(audT_sb[:, ci, :], tp[:])

            # ---- qT = Wq.T @ imgT, scaled + bias ----
            qT_sb = big_pool.tile([P, NCA, HW], MM, tag="qT", bufs=2)
            for m in range(NCA):
                for n in range(NQ):
                    ps = fr_ps.tile([P, NT], F32, tag="fr")
                    for ci in range(NCI):
                        nc.tensor.matmul(
                            ps[:],
                            wq_sb[:, ci, m * P:(m + 1) * P],
                            img_sb[:, ci, n * NT:(n + 1) * NT],
                            start=(ci == 0), stop=(ci == NCI - 1),
                        )
                    nc.scalar.activation(qT_sb[:, m, n * NT:(n + 1) * NT], ps[:], Ident,
                                         bias=bq_col[:, m:m + 1], scale=SCALE)
                if dbg and b == 0:
                    dump_f32(dbg_d["qT"][m * P:(m + 1) * P, :], ff(qT_sb[:, m, :]))

            # ---- kT = Wk.T @ audT + bk ----
            kT_sb = kv_pool.tile([P, NCA, K_LEN], MM, tag="kT", bufs=2)
            for m in range(NCA):
                ps = fr_ps.tile([P, K_LEN], F32, tag="fr")
                for ci in range(NCA):
                    nc.tensor.matmul(
                        ps[:],
                        wk_sb[:, ci, m * P:(m + 1) * P],
                        audT_sb[:, ci, :],
                        start=(ci == 0), stop=(ci == NCA - 1),
                    )
                nc.scalar.activation(kT_sb[:, m, :], ps[:], Ident, bias=bk_col[:, m:m + 1])
                if dbg and b == 0:
                    dump_f32(dbg_d["kT"][m * P:(m + 1) * P, :], ff(kT_sb[:, m, :]))

            # ---- v = audT.T @ Wv + bv (seq-major) ----
            v_sb = kv_pool.tile([P, NS, C_AUD], BF16, tag="v", bufs=2)
            for st in range(NS):
                ps = fr_ps.tile([P, C_AUD], F32, tag="fr")
                for ci in range(NCA):
                    nc.tensor.matmul(
                        ps[:],
                        audT_sb[:, ci, st * P:(st + 1) * P],
                        wv_sb[:, ci, :],
                        start=(ci == 0), stop=False,
                    )
                nc.tensor.matmul(ps[:], ones_row[:], bv_row[:],
                                 start=False, stop=True)
                nc.vector.tensor_copy(v_sb[:, st, :], ps[:])
                if dbg and b == 0:
                    dump_f32(dbg_d["v"][st * P:(st + 1) * P, :], v_sb[:, st, :])

            # ---- attention, head pairs (2t, 2t+1) ----
            attnT_sb = big_pool.tile([P, NCA, HW], MM, tag="attnT")
            for t in range(N_HEADS // 2):
                expS = []  # [hh][kt] sbuf tiles [128, HW]
                for hh in range(2):
                    h = 2 * t + hh
                    ht, hr = h // 2, (h % 2) * HD
                    eh = []
                    for kt in range(NS):
                        et = expS_pool.tile([P, HW], BF16, tag="expS")
                        for n in range(NQ):
                            sps = mm_ps.tile([P, NT], F32, tag="ps")
                            nc.tensor.matmul(
                                sps[:],
                                kT_sb[hr:hr + HD, ht, kt * P:(kt + 1) * P],
                                qT_sb[hr:hr + HD, ht, n * NT:(n + 1) * NT],
                                start=True, stop=True,
                            )
                            nc.scalar.activation(et[:, n * NT:(n + 1) * NT], sps[:], Exp)
                        if dbg and b == 0 and t == 0:
                            dump_f32(dbg_d[f"expS{hh}"][kt * P:(kt + 1) * P, :], et[:])
                        eh.append(et)
                    expS.append(eh)

                # attn (col-tiled pairs) + replicated sumexp rows (matmul with an
                # all-ones stationary does the reduction AND the partition
                # broadcast in one shot -> everything stays lane-aligned)
                apss, sebcs = [], []
                for n in range(NQ):
                    aps = mm_ps.tile([P, NT], F32, tag="ps")
                    sebc = mm_ps.tile([P, NT], F32, tag="ps")
                    for hh in range(2):
                        h = 2 * t + hh
                        r0 = hh * HD
                        for kt in range(NS):
                            nc.tensor.matmul(
                                aps[r0:r0 + HD, :],
                                v_sb[:, kt, h * HD:(h + 1) * HD],
                                expS[hh][kt][:, n * NT:(n + 1) * NT],
                                start=(kt == 0), stop=(kt == NS - 1),
                                tile_position=(0, r0),
                            )
                            nc.tensor.matmul(
                                sebc[r0:r0 + HD, :],
                                ones_bf64[:],
                                expS[hh][kt][:, n * NT:(n + 1) * NT],
                                start=(kt == 0), stop=(kt == NS - 1),
                                tile_position=(0, r0),
                            )
                    apss.append(aps)
                    sebcs.append(sebc)

                # normalize: attnT_chunk = attn_pair_psum * recip(sumexp_bcast)
                rb = rb_pool.tile([P, HW], F32, tag="rb")
                for n in range(NQ):
                    nc.vector.reciprocal(rb[:, n * NT:(n + 1) * NT], sebcs[n][:])
                if dbg and b == 0 and t == 0:
                    for n in range(NQ):
                        dump_f32(dbg_d["sumexp"][:, n * NT:(n + 1) * NT], sebcs[n][:])
                        dump_f32(dbg_d["anum"][:, n * NT:(n + 1) * NT], apss[n][:])
                    dump_f32(dbg_d["rb"][:, :], rb[:])
                for n in range(NQ):
                    nc.vector.tensor_tensor(attnT_sb[:, t, n * NT:(n + 1) * NT],
                                            apss[n][:], rb[:, n * NT:(n + 1) * NT], MUL)
                if dbg and b == 0:
                    dump_f32(dbg_d["attnT"][t * P:(t + 1) * P, :], ff(attnT_sb[:, t, :]))

            # ---- projT = Wo.T @ attnT + bo ; y = imgT + projT ----
            # reuse qT's slot: qT is fully consumed by the score matmuls above
            y_sb = big_pool.tile([P, NCI, HW], MM, tag="qT", bufs=2)
            for m in range(NCI):
                for n in range(NQ):
                    ps = mm_ps.tile([P, NT], F32, tag="ps")
                    for ci in range(NCA):
                        nc.tensor.matmul(
                            ps[:],
                            wo_sb[:, ci, m * P:(m + 1) * P],
                            attnT_sb[:, ci, n * NT:(n + 1) * NT],
                            start=(ci == 0), stop=False,
                        )
                    # residual: += I.T @ imgT folds y = proj + img into the psum
                    nc.tensor.matmul(ps[:], ident_r[:],
                                     img_sb[:, m, n * NT:(n + 1) * NT],
                                     start=False, stop=True)
                    nc.scalar.activation(y_sb[:, m, n * NT:(n + 1) * NT], ps[:], Ident,
                                         bias=bo_col[:, m:m + 1])
                if dbg and b == 0:
                    dump_f32(dbg_d["y"][m * P:(m + 1) * P, :], ff(y_sb[:, m, :]))

            # ---- layernorm stats over C via ones-matmuls (both rows at partition 0) ----
            sum_ps_n = [mm_ps.tile([1, NT], F32, tag="ps", name=f"sum_ps{b}_{i}") for i in range(NQ)]
            for ci in range(NCI):
                for n in range(NQ):
                    nc.tensor.matmul(
                        sum_ps_n[n][:], ones_col[:], y_sb[:, ci, n * NT:(n + 1) * NT],
                        start=(ci == 0), stop=(ci == NCI - 1),
                    )
            sq_ps_n = [mm_ps.tile([1, NT], F32, tag="ps", name=f"sq_ps{b}_{i}") for i in range(NQ)]
            for ci in range(NCI):
                ysq = chunk_pool.tile([P, HW], MM, tag="chunk")
                nc.vector.tensor_tensor(ysq[:], ff(y_sb[:, ci, :]), ff(y_sb[:, ci, :]), MUL)
                for n in range(NQ):
                    nc.tensor.matmul(
                        sq_ps_n[n][:], ones_col[:], ysq[:, n * NT:(n + 1) * NT],
                        start=(ci == 0), stop=(ci == NCI - 1),
                    )

            mean = row1_pool.tile([1, HW], F32, tag="r1")
            e2 = row1_pool.tile([1, HW], F32, tag="r1")
            for n in range(NQ):
                nc.scalar.activation(mean[:, n * NT:(n + 1) * NT], sum_ps_n[n][:],
                                     Copy, scale=1.0 / C_IMG)
                nc.scalar.activation(e2[:, n * NT:(n + 1) * NT], sq_ps_n[n][:],
                                     Copy, scale=1.0 / C_IMG)
            mu2 = row1_pool.tile([1, HW], F32, tag="r1")
            nc.vector.tensor_tensor(mu2[:], mean[:], mean[:], MUL)
            nmu = row1_pool.tile([1, HW], F32, tag="r1")
            nc.vector.tensor_scalar_mul(nmu[:], mean[:], -1.0)
            var = row1_pool.tile([1, HW], F32, tag="r1")
            nc.vector.tensor_tensor(var[:], e2[:], mu2[:], SUB)
            std = row1_pool.tile([1, HW], F32, tag="r1")
            nc.scalar.activation(std[:], var[:], Sqrt, bias=eps_col[:])
            rstd = row1_pool.tile([1, HW], F32, tag="r1")
            nc.vector.reciprocal(rstd[:], std[:])
            # rows in MM dtype for the K=1 broadcast matmuls
            nmu_r = row1_pool.tile([1, HW], MM, tag="r1")
            nc.scalar.activation(nmu_r[:], nmu[:], Copy)
            rstd_r = row1_pool.tile([1, HW], MM, tag="r1")
            nc.scalar.activation(rstd_r[:], rstd[:], Copy)

            # broadcast -mu and rstd across partitions via K=1 ones matmuls,
            # evacuated to SBUF so the apply ops run in DVE 2x (SBUF-only) mode
            nmu_bc = rb_pool.tile([P, HW], F32, tag="rb")
            rstd_bc = rb_pool.tile([P, HW], F32, tag="rb")
            for n in range(NQ):
                bc1 = mm_ps.tile([P, NT], F32, tag="ps")
                nc.tensor.matmul(bc1[:], ones_row[:],
                                 nmu_r[:, n * NT:(n + 1) * NT], start=True, stop=True)
                nc.vector.tensor_copy(nmu_bc[:, n * NT:(n + 1) * NT], bc1[:])
                bc2 = mm_ps.tile([P, NT], F32, tag="ps")
                nc.tensor.matmul(bc2[:], ones_row[:],
                                 rstd_r[:, n * NT:(n + 1) * NT], start=True, stop=True)
                nc.vector.tensor_copy(rstd_bc[:, n * NT:(n + 1) * NT], bc2[:])

            # ---- apply: out = gamma * (y - mu) * rstd + beta ----
            for ci in range(NCI):
                t1 = chunk_pool.tile([P, HW], F32, tag="chunk")
                nc.vector.tensor_tensor(t1[:], ff(y_sb[:, ci, :]), nmu_bc[:], ADD)
                t2 = chunk_pool.tile([P, HW], F32, tag="chunk")
                nc.vector.tensor_tensor(t2[:], t1[:], rstd_bc[:], MUL)
                o = chunk_pool.tile([P, HW], F32, tag="chunk")
                nc.vector.tensor_scalar(out=o[:], in0=t2[:],
                                        scalar1=gam_col[:, ci:ci + 1],
                                        scalar2=bet_col[:, ci:ci + 1],
                                        op0=MUL, op1=ADD)
                nc.sync.dma_start(out=out_d[b, ci * P:(ci + 1) * P, :], in_=o[:])


def build(mm_mode=MM_MODE, dbg=False, repeat=1):
    mm_dt = mybir.dt.float32r if mm_mode == "f32r" else F32
    nc = bacc.Bacc("TRN2", target_bir_lowering=False, debug=False)
    with tile.TileContext(nc) as tc, ExitStack() as ctx:
        _body(ctx, tc, mm_dt, dbg=dbg, repeat=repeat)
    nc.compile()
    return nc


def build_debug(mm_mode=MM_MODE):
    return build(mm_mode, dbg=True)


_NC_CACHE = {}


def _get_nc(mm_mode=MM_MODE):
    if mm_mode not in _NC_CACHE:
        _NC_CACHE[mm_mode] = build(mm_mode)
    return _NC_CACHE[mm_mode]


def _in_maps(inputs):
    img = np.ascontiguousarray(np.asarray(inputs["img_feat"], np.float32)
                               .reshape(B, C_IMG, HW))
    aud = np.ascontiguousarray(np.asarray(inputs["audio_feat"], np.float32))
    shared = {
        "wq": np.asarray(inputs["Wq"], np.float32),
        "wk": np.asarray(inputs["Wk"], np.float32),
        "wv": np.asarray(inputs["Wv"], np.float32),
        "wo": np.asarray(inputs["Wo"], np.float32),
        "bq": np.asarray(inputs["bq"], np.float32),
        "bk": np.asarray(inputs["bk"], np.float32),
        "bv": np.asarray(inputs["bv"], np.float32),
        "bo": np.asarray(inputs["bo"], np.float32),
        "gamma": np.asarray(inputs["gamma"], np.float32),
        "beta": np.asarray(inputs["beta"], np.float32),
    }
    maps = []
    for c in range(N_CORES):
        sl = slice(c * BPC, (c + 1) * BPC)
        maps.append({"img": img[sl], "aud": aud[sl], **shared})
    return maps


def kernel(**inputs) -> np.ndarray:
    nc = _get_nc()
    res = run_bass_kernel_spmd(nc, _in_maps(inputs), list(range(N_CORES)))
    outs = [res.results[c]["out"] for c in range(N_CORES)]
    return np.concatenate(outs, axis=0).reshape(B, C_IMG, H, W)


def kernel_profiled(inputs, mm_mode=MM_MODE, **kw):
    """Returns (output, BassKernelResults). NTFF tracing is unavailable in this
    container (axon.trn not shipped), so exec_time_ns is None; use test.py's
    repeated-call timing instead."""
    nc = _get_nc(mm_mode)
    res = run_bass_kernel_spmd(nc, _in_maps(inputs), list(range(N_CORES)), **kw)
    outs = [res.results[c]["out"] for c in range(N_CORES)]
    return np.concatenate(outs, axis=0).reshape(B, C_IMG, H, W), res



# revision 7
# speedup vs baseline: 1336.7835x; 1336.7835x over previous
"""Fused cross-attention audio fuser (dense transformer block) on TRN2.

Strategy: batch data-parallel across 8 NeuronCores (B=16 -> 2 batches/core,
no collectives), with a q-major attention pipeline in fp8/bf16:

  kT = Wk.T @ audT (+bk via Act-evac bias)       [C_AUD, K_LEN]  fp8
  v  = audT.T @ Wv (+bv via K=1 matmul)          [K_LEN, C_AUD]  fp8,
       stored per-head with a fused ones column ([K, h, 65])
  qT = Wq.T @ imgT (+bq via Act-evac bias)       [C_AUD, HW]     fp8
  S_h = kT_h.T @ qT_h; expS = exp(SCALE*S)       [K_LEN, HW]     fp8
  attn_q[q, h, d|sum] = expS_h.T @ [v_h | 1]     q-major, the ones column
       of v makes column 64 the softmax denominator for free
  attn_q normalized by per-partition (=per-q) reciprocal (DVE broadcast AP)
  attnT = dma_transpose(attn_q)                  [C_AUD, HW]     bf16
  projT = Wo.T @ attnT + I.T @ (img + bo)        residual+bias fused in psum
  layernorm: stats via (1/C)*ones matmuls (sum at psum partition 0, sum of
       squares at partition 32 of the same bank), Rsqrt row, ones-matmul
       partition broadcast, 3-pass bf16 DVE apply with per-partition
       gamma/beta.

All matmul operands fp8e4 or bf16 (1 cyc/col); exp scale folds the attention
scale; bo is folded into the residual image host-side; bq/bk ride Act
evacuation bias slots; bv rides a K=1 ones matmul (fp8-quantized, exact for
bv=0).
"""

import numpy as np
from contextlib import ExitStack

import concourse.bass as bass
import concourse.mybir as mybir
import concourse.tile as tile
from concourse import bacc
from concourse.bass_utils import run_bass_kernel_spmd
from concourse.masks import make_identity

# Problem constants (hardcoded per spec)
B, C_IMG, H, W = 16, 512, 32, 32
C_AUD, K_LEN, N_HEADS = 512, 256, 8
HD = C_AUD // N_HEADS           # 64
HW = H * W                      # 1024
EPS = 1e-5
SCALE = float(HD) ** -0.5       # 0.125
N_CORES = 8
BPC = B // N_CORES              # 2 batches per core

F32 = mybir.dt.float32
F32R = mybir.dt.float32r
BF16 = mybir.dt.bfloat16
FP8 = mybir.dt.float8e4
P = 128
NCI = C_IMG // P                # 4 img-channel chunks
NCA = C_AUD // P                # 4 audio-channel chunks
NS = K_LEN // P                 # 2 key chunks
NT = 512                        # matmul free tile
NQ = HW // NT                   # 2 q column tiles
QC = HW // P                    # 8 q chunks of 128 (partition dim in q-major)
HB = N_HEADS // 2               # head pairs

MM_MODE = "fp8"                 # informational only

Ident = mybir.ActivationFunctionType.Identity
Copy = mybir.ActivationFunctionType.Copy
Exp = mybir.ActivationFunctionType.Exp
Sqrt = mybir.ActivationFunctionType.Sqrt
Square = mybir.ActivationFunctionType.Square
ADD = mybir.AluOpType.add
SUB = mybir.AluOpType.subtract
MUL = mybir.AluOpType.mult


def _body(ctx: ExitStack, tc: tile.TileContext, dbg=False, repeat=1):
    nc = tc.nc

    audt_d = nc.dram_tensor("audt", [BPC, NCA, P, K_LEN], FP8, kind="ExternalInput").ap()
    img8_d = nc.dram_tensor("img8", [BPC, NCI, P, HW], FP8, kind="ExternalInput").ap()
    imgb_d = nc.dram_tensor("imgb", [BPC, NCI, P, HW], BF16, kind="ExternalInput").ap()
    wq_d = nc.dram_tensor("wq", [NCI, P, C_AUD], FP8, kind="ExternalInput").ap()
    wk_d = nc.dram_tensor("wk", [NCA, P, C_AUD], FP8, kind="ExternalInput").ap()
    wv_d = nc.dram_tensor("wv", [NCA, P, C_AUD], FP8, kind="ExternalInput").ap()
    wo_d = nc.dram_tensor("wo", [NCA, P, C_IMG], FP8, kind="ExternalInput").ap()
    bq_d = nc.dram_tensor("bq", [C_AUD], F32, kind="ExternalInput").ap()
    bk_d = nc.dram_tensor("bk", [C_AUD], F32, kind="ExternalInput").ap()
    bv_d = nc.dram_tensor("bv", [C_AUD], FP8, kind="ExternalInput").ap()
    gam_d = nc.dram_tensor("gamma", [C_IMG], F32, kind="ExternalInput").ap()
    bet_d = nc.dram_tensor("beta", [C_IMG], F32, kind="ExternalInput").ap()
    out_d = nc.dram_tensor("outb", [BPC, NCI, P, HW], BF16, kind="ExternalOutput").ap()

    if dbg:
        dbg_d = {
            "qT": nc.dram_tensor("dbg_qT", [NCA, P, HW], F32, kind="ExternalOutput").ap(),
            "kT": nc.dram_tensor("dbg_kT", [NCA, P, K_LEN], F32, kind="ExternalOutput").ap(),
            "v8": nc.dram_tensor("dbg_v8", [NS, P, 8 * 65], F32, kind="ExternalOutput").ap(),
            "expS": nc.dram_tensor("dbg_expS", [NS, P, HW], F32, kind="ExternalOutput").ap(),
            "attnq": nc.dram_tensor("dbg_attnq", [QC, P, C_AUD], F32, kind="ExternalOutput").ap(),
            "attnT": nc.dram_tensor("dbg_attnT", [NCA, P, HW], F32, kind="ExternalOutput").ap(),
            "y": nc.dram_tensor("dbg_y", [NCI, P, HW], F32, kind="ExternalOutput").ap(),
        }
        dbg_pool = ctx.enter_context(tc.tile_pool(name="dbgp", bufs=2))

        def dump(dst, src_ap, rows=P):
            t = dbg_pool.tile([rows, src_ap.shape[-1]], F32, tag="dbg")
            nc.scalar.activation(t[:], src_ap, Copy)
            nc.sync.dma_start(out=dst, in_=t[:])

    # ---- pools ----
    cpool = ctx.enter_context(tc.tile_pool(name="consts", bufs=1))
    wpool = ctx.enter_context(tc.tile_pool(name="weights", bufs=1))
    inp_pool = ctx.enter_context(tc.tile_pool(name="inp", bufs=2))
    kv_pool = ctx.enter_context(tc.tile_pool(name="kv", bufs=2))
    qT_pool = ctx.enter_context(tc.tile_pool(name="qT", bufs=2))
    expS_pool = ctx.enter_context(tc.tile_pool(name="expS", bufs=9))
    aq_pool = ctx.enter_context(tc.tile_pool(name="aq", bufs=2))
    y_pool = ctx.enter_context(tc.tile_pool(name="y", bufs=2))
    row_pool = ctx.enter_context(tc.tile_pool(name="rows", bufs=4))
    bc_pool = ctx.enter_context(tc.tile_pool(name="bc", bufs=2))
    chunk_pool = ctx.enter_context(tc.tile_pool(name="chunk", bufs=4))
    mm_ps = ctx.enter_context(tc.tile_pool(name="mm_ps", bufs=2, space="PSUM"))
    sc_ps = ctx.enter_context(tc.tile_pool(name="sc_ps", bufs=2, space="PSUM"))
    at_ps = ctx.enter_context(tc.tile_pool(name="at_ps", bufs=2, space="PSUM"))
    st_ps = mm_ps  # stats psums time-share the mm slots (same tag below)

    # ---- constants ----
    ones_f32 = cpool.tile([P, P], F32, tag="ones_f32")
    nc.vector.memset(ones_f32[:], 1.0)
    ident_f32 = cpool.tile([P, P], F32, tag="ident")
    make_identity(nc, ident_f32[:])
    identb = cpool.tile([P, P], BF16, tag="identb")
    nc.scalar.activation(identb[:], ident_f32[:], Copy)
    ones_row8 = cpool.tile([1, P], FP8, tag="ones_row8")
    nc.scalar.activation(ones_row8[:], ones_f32[0:1, :], Copy)
    ones_rowb = cpool.tile([1, P], BF16, tag="ones_rowb")
    nc.scalar.activation(ones_rowb[:], ones_f32[0:1, :], Copy)
    oneC_col = cpool.tile([P, 1], BF16, tag="oneC")   # 1/C_IMG for LN stats
    invc = cpool.tile([P, 1], F32, tag="invc")
    nc.vector.memset(invc[:], 1.0 / C_IMG)
    nc.scalar.activation(oneC_col[:], invc[:], Copy)
    eps_col = cpool.tile([1, 1], F32, tag="eps")
    nc.vector.memset(eps_col[:], EPS)

    wq_sb = wpool.tile([P, NCI, C_AUD], FP8, tag="wq")
    wk_sb = wpool.tile([P, NCA, C_AUD], FP8, tag="wk")
    wv_sb = wpool.tile([P, NCA, C_AUD], FP8, tag="wv")
    wo_sb = wpool.tile([P, NCA, C_IMG], FP8, tag="wo")
    bq_col = cpool.tile([P, NCA], F32, tag="bq")
    bk_col = cpool.tile([P, NCA], F32, tag="bk")
    bv_row = cpool.tile([1, C_AUD], FP8, tag="bv")
    gam_col = cpool.tile([P, NCI], F32, tag="gam")
    bet_col = cpool.tile([P, NCI], F32, tag="bet")

    for rep in range(repeat):
        # ---- input + weight DMAs (ordered to feed the first consumers) ----
        aud_tiles, img8_tiles, imgb_tiles = [], [], []
        for b in range(BPC):
            aud_tiles.append(inp_pool.tile([P, NCA, K_LEN], FP8, tag="aud", name=f"aud{b}"))
            img8_tiles.append(inp_pool.tile([P, NCI, HW], FP8, tag="img8", name=f"img8_{b}"))
            imgb_tiles.append(inp_pool.tile([P, NCI, HW], BF16, tag="imgb", name=f"imgb{b}"))
        for ci in range(NCA):
            nc.sync.dma_start(out=wk_sb[:, ci, :], in_=wk_d[ci])
        for b in range(BPC):
            for ci in range(NCA):
                nc.sync.dma_start(out=aud_tiles[b][:, ci, :], in_=audt_d[b, ci])
        for m in range(NCA):
            nc.sync.dma_start(out=bk_col[:, m:m + 1], in_=bk_d[m * P:(m + 1) * P])
            nc.sync.dma_start(out=bq_col[:, m:m + 1], in_=bq_d[m * P:(m + 1) * P])
        for ci in range(NCA):
            nc.sync.dma_start(out=wv_sb[:, ci, :], in_=wv_d[ci])
        nc.sync.dma_start(out=bv_row[:], in_=bv_d[:])
        for ci in range(NCI):
            nc.sync.dma_start(out=wq_sb[:, ci, :], in_=wq_d[ci])
        for b in range(BPC):
            for ci in range(NCI):
                nc.sync.dma_start(out=img8_tiles[b][:, ci, :], in_=img8_d[b, ci])
        for ci in range(NCA):
            nc.sync.dma_start(out=wo_sb[:, ci, :], in_=wo_d[ci])
        for b in range(BPC):
            for ci in range(NCI):
                nc.sync.dma_start(out=imgb_tiles[b][:, ci, :], in_=imgb_d[b, ci])
        for m in range(NCI):
            nc.sync.dma_start(out=gam_col[:, m:m + 1], in_=gam_d[m * P:(m + 1) * P])
            nc.sync.dma_start(out=bet_col[:, m:m + 1], in_=bet_d[m * P:(m + 1) * P])

        # ---- per-batch state ----
        kT8 = [None] * BPC      # [128, NCA, K_LEN] fp8, head-pair-major
        v8 = [None] * BPC       # [128, NS, 8*65] fp8 (65th col of each head = 1)
        qT8 = [None] * BPC      # [128, NCA, HW] fp8
        expS8 = [None] * BPC    # per head: [128, NS, HW] fp8
        aqbf = [None] * BPC     # [128, QC, C_AUD] bf16 (q-major attn)
        attnT = [None] * BPC    # [128, NCA, HW] bf16
        ybf = [None] * BPC      # [128, NCI, HW] bf16
        nmu_bf = [None] * BPC
        rstd_bf = [None] * BPC

        def ph_kT(b):
            kT8[b] = kv_pool.tile([P, NCA, K_LEN], FP8, tag="kT", name=f"kT{b}")
            for m in range(NCA):
                ps = mm_ps.tile([P, K_LEN], F32, tag="mm")
                for ci in range(NCA):
                    nc.tensor.matmul(ps[:], wk_sb[:, ci, m * P:(m + 1) * P],
                                     aud_tiles[b][:, ci, :],
                                     start=(ci == 0), stop=(ci == NCA - 1))
                nc.scalar.activation(kT8[b][:, m, :], ps[:], Ident,
                                     bias=bk_col[:, m:m + 1])
                if dbg and b == 0:
                    dump(dbg_d["kT"][m], kT8[b][:, m, :])

        def ph_v(b):
            v8[b] = kv_pool.tile([P, NS, 8 * 65], FP8, tag="v8", name=f"v8_{b}")
            # ones columns (col 64 of each head block)
            nc.scalar.activation(
                v8[b][:].rearrange("p k (h x) -> p k h x", h=8)[:, :, :, 64],
                ones_f32[:, 0:16].rearrange("p (k h) -> p k h", k=NS), Copy)
            for kt in range(NS):
                ps = mm_ps.tile([P, C_AUD], F32, tag="mm")
                for ci in range(NCA):
                    nc.tensor.matmul(ps[:], aud_tiles[b][:, ci, kt * P:(kt + 1) * P],
                                     wv_sb[:, ci, :],
                                     start=(ci == 0), stop=False)
                nc.tensor.matmul(ps[:], ones_row8[:], bv_row[:],
                                 start=False, stop=True)
                nc.scalar.activation(
                    v8[b][:, kt, :].rearrange("p (h x) -> p h x", h=8)[:, :, 0:64],
                    ps[:].rearrange("p (h x) -> p h x", h=8), Copy)
                if dbg and b == 0:
                    dump(dbg_d["v8"][kt], v8[b][:, kt, :])

        def ph_qT(b):
            qT8[b] = qT_pool.tile([P, NCA, HW], FP8, tag="qT", name=f"qT{b}")
            for m in range(NCA):
                for n in range(NQ):
                    ps = mm_ps.tile([P, NT], F32, tag="mm")
                    for ci in range(NCI):
                        nc.tensor.matmul(ps[:], wq_sb[:, ci, m * P:(m + 1) * P],
                                         img8_tiles[b][:, ci, n * NT:(n + 1) * NT],
                                         start=(ci == 0), stop=(ci == NCI - 1))
                    nc.scalar.activation(qT8[b][:, m, n * NT:(n + 1) * NT], ps[:],
                                         Ident, bias=bq_col[:, m:m + 1])
                if dbg and b == 0:
                    dump(dbg_d["qT"][m], qT8[b][:, m, :])

        def ph_se(b):
            expS8[b] = []
            for h in range(N_HEADS):
                ht, hr = h // 2, (h % 2) * HD
                et = expS_pool.tile([P, NS, HW], FP8, tag="expS", name=f"expS{b}_{h}")
                for kt in range(NS):
                    ps = sc_ps.tile([P, HW], F32, tag="sc")
                    for n in range(NQ):
                        nc.tensor.matmul(
                            ps[:, n * NT:(n + 1) * NT],
                            kT8[b][hr:hr + HD, ht, kt * P:(kt + 1) * P],
                            qT8[b][hr:hr + HD, ht, n * NT:(n + 1) * NT],
                            start=True, stop=True)
                    nc.scalar.activation(et[:, kt, :], ps[:], Exp, scale=SCALE)
                    if dbg and b == 0 and h == 0:
                        dump(dbg_d["expS"][kt], et[:, kt, :])
                expS8[b].append(et)

        def ph_attn(b):
            aqbf[b] = aq_pool.tile([P, QC, C_AUD], BF16, tag="aq", name=f"aq{b}")
            attnT[b] = aq_pool.tile([P, NCA, HW], BF16, tag="attnT", name=f"aT{b}")
            for qc in range(QC):
                for half in range(2):
                    pa = at_ps.tile([P, 4, 65], F32, tag="at")
                    for hh in range(4):
                        h = half * 4 + hh
                        for kt in range(NS):
                            nc.tensor.matmul(
                                pa[:, hh, :],
                                expS8[b][h][:, kt, qc * P:(qc + 1) * P],
                                v8[b][:, kt, h * 65:(h + 1) * 65],
                                start=(kt == 0), stop=(kt == NS - 1))
                    rq = chunk_pool.tile([P, 4], F32, tag="rq")
                    nc.vector.reciprocal(rq[:], pa[:, :, 64])
                    nc.vector.tensor_tensor(
                        aqbf[b][:, qc, half * 256:(half + 1) * 256]
                        .rearrange("p (h x) -> p h x", h=4),
                        pa[:, :, 0:64],
                        rq[:].unsqueeze(-1).broadcast_to([P, 4, 64]), MUL)
                # transpose this q-chunk to channel-major
                nc.sync.dma_start_transpose(
                    attnT[b][:, :, qc * P:(qc + 1) * P], aqbf[b][:, qc, :])
                if dbg and b == 0:
                    dump(dbg_d["attnq"][qc], aqbf[b][:, qc, :])
            if dbg and b == 0:
                for ci in range(NCA):
                    dump(dbg_d["attnT"][ci], attnT[b][:, ci, :])

        def ph_proj(b):
            ybf[b] = y_pool.tile([P, NCI, HW], BF16, tag="y", name=f"y{b}")
            for m in range(NCI):
                for n in range(NQ):
                    ps = mm_ps.tile([P, NT], F32, tag="mm")
                    for ca in range(NCA):
                        nc.tensor.matmul(ps[:], wo_sb[:, ca, m * P:(m + 1) * P],
                                         attnT[b][:, ca, n * NT:(n + 1) * NT],
                                         start=(ca == 0), stop=False)
                    nc.tensor.matmul(ps[:], identb[:],
                                     imgb_tiles[b][:, m, n * NT:(n + 1) * NT],
                                     start=False, stop=True, skip_group_check=True)
                    nc.vector.tensor_copy(ybf[b][:, m, n * NT:(n + 1) * NT], ps[:])
                if dbg and b == 0:
                    dump(dbg_d["y"][m], ybf[b][:, m, :])

        def ph_stats(b):
            stp = [st_ps.tile([64, NT], F32, tag="mm", name=f"st{b}_{n}")
                   for n in range(NQ)]
            for ci in range(NCI):
                for n in range(NQ):
                    nc.tensor.matmul(stp[n][0:1, :], oneC_col[:],
                                     ybf[b][:, ci, n * NT:(n + 1) * NT],
                                     start=(ci == 0), stop=(ci == NCI - 1))
            for ci in range(NCI):
                ysq = chunk_pool.tile([P, HW], BF16, tag="ysq")
                nc.vector.tensor_tensor(ysq[:], ybf[b][:, ci, :], ybf[b][:, ci, :], MUL)
                for n in range(NQ):
                    nc.tensor.matmul(stp[n][32:33, :], oneC_col[:],
                                     ysq[:, n * NT:(n + 1) * NT],
                                     start=(ci == 0), stop=(ci == NCI - 1))
            # rows: mean at partition 0, E[y^2] at partition 32
            nmu_bf[b] = row_pool.tile([1, HW], BF16, tag="row", name=f"nmu{b}")
            rstd_bf[b] = row_pool.tile([1, HW], BF16, tag="row", name=f"rstd{b}")
            for n in range(NQ):
                sl = slice(n * NT, (n + 1) * NT)
                mu2 = row_pool.tile([1, NT], F32, tag="mu2")
                nc.scalar.activation(mu2[:], stp[n][0:1, :], Square)
                var = row_pool.tile([1, NT], F32, tag="var")
                nc.vector.tensor_tensor(var[:], stp[n][32:33, :], mu2[:], SUB)
                std = row_pool.tile([1, NT], F32, tag="std")
                nc.scalar.activation(std[:], var[:], Sqrt, bias=eps_col[:])
                with nc.allow_low_precision(reason="bf16 rstd row; |err|<2^-9 ok"):
                    nc.vector.reciprocal(rstd_bf[b][:, sl], std[:])
                nc.vector.tensor_scalar_mul(nmu_bf[b][:, sl], stp[n][0:1, :], -1.0)
            # partition broadcasts via K=1 ones matmuls
            nbc = bc_pool.tile([P, HW], BF16, tag="nbc", name=f"nbc{b}")
            rbc = bc_pool.tile([P, HW], BF16, tag="rbc", name=f"rbc{b}")
            for n in range(NQ):
                sl = slice(n * NT, (n + 1) * NT)
                b1 = mm_ps.tile([P, NT], F32, tag="mm")
                nc.tensor.matmul(b1[:], ones_rowb[:], nmu_bf[b][:, sl],
                                 start=True, stop=True)
                nc.vector.tensor_copy(nbc[:, sl], b1[:])
                b2 = mm_ps.tile([P, NT], F32, tag="mm")
                nc.tensor.matmul(b2[:], ones_rowb[:], rstd_bf[b][:, sl],
                                 start=True, stop=True)
                nc.vector.tensor_copy(rbc[:, sl], b2[:])
            nmu_bf[b], rstd_bf[b] = nbc, rbc

        def ph_apply(b):
            for ci in range(NCI):
                t1 = chunk_pool.tile([P, HW], BF16, tag="t1")
                nc.vector.tensor_tensor(t1[:], ybf[b][:, ci, :], nmu_bf[b][:], ADD)
                t2 = chunk_pool.tile([P, HW], BF16, tag="t2")
                nc.vector.tensor_tensor(t2[:], t1[:], rstd_bf[b][:], MUL)
                o = chunk_pool.tile([P, HW], BF16, tag="o")
                nc.vector.tensor_scalar(out=o[:], in0=t2[:],
                                        scalar1=gam_col[:, ci:ci + 1],
                                        scalar2=bet_col[:, ci:ci + 1],
                                        op0=MUL, op1=ADD)
                nc.sync.dma_start(out=out_d[b, ci], in_=o[:])

        # ---- phase schedule (emission order == per-engine execution order) ----
        for b in range(BPC):
            ph_kT(b)
        for b in range(BPC):
            ph_v(b)
        for b in range(BPC):
            ph_qT(b)
        ph_se(0)
        ph_attn(0)
        ph_se(1)
        ph_attn(1)
        ph_proj(0)
        ph_stats(0)
        ph_proj(1)
        ph_stats(1)
        ph_apply(0)
        ph_apply(1)


def build(mm_mode=MM_MODE, dbg=False, repeat=1):
    nc = bacc.Bacc("TRN2", target_bir_lowering=False, debug=False)
    with tile.TileContext(nc) as tc, ExitStack() as ctx:
        _body(ctx, tc, dbg=dbg, repeat=repeat)
    nc.compile()
    return nc


def build_debug(mm_mode=MM_MODE):
    return build(mm_mode, dbg=True)


_NC_CACHE = {}


def _get_nc(mm_mode=MM_MODE):
    if mm_mode not in _NC_CACHE:
        _NC_CACHE[mm_mode] = build(mm_mode)
    return _NC_CACHE[mm_mode]


NP8 = mybir.dt.np(FP8)
NPBF = mybir.dt.np(BF16)


def _in_maps(inputs):
    img = np.asarray(inputs["img_feat"], np.float32).reshape(B, C_IMG, HW)
    aud = np.asarray(inputs["audio_feat"], np.float32)
    bo = np.asarray(inputs["bo"], np.float32)

    # audT: [B, K, C] -> [B, C, K] -> chunked [B, NCA, 128, K]
    audt = np.ascontiguousarray(aud.transpose(0, 2, 1)).reshape(B, NCA, P, K_LEN)
    img8 = img.reshape(B, NCI, P, HW)
    imgb = (img + bo[None, :, None]).reshape(B, NCI, P, HW)

    def wprep(w):
        return np.ascontiguousarray(np.asarray(w, np.float32)
                                    .reshape(NCA, P, -1)).astype(NP8)

    shared = {
        "wq": wprep(inputs["Wq"]),
        "wk": wprep(inputs["Wk"]),
        "wv": wprep(inputs["Wv"]),
        "wo": wprep(inputs["Wo"]),
        "bq": np.asarray(inputs["bq"], np.float32),
        "bk": np.asarray(inputs["bk"], np.float32),
        "bv": np.asarray(inputs["bv"], np.float32).astype(NP8),
        "gamma": np.asarray(inputs["gamma"], np.float32),
        "beta": np.asarray(inputs["beta"], np.float32),
    }
    maps = []
    for c in range(N_CORES):
        sl = slice(c * BPC, (c + 1) * BPC)
        maps.append({
            "audt": audt[sl].astype(NP8),
            "img8": img8[sl].astype(NP8),
            "imgb": imgb[sl].astype(NPBF),
            **shared,
        })
    return maps


def kernel(**inputs) -> np.ndarray:
    nc = _get_nc()
    res = run_bass_kernel_spmd(nc, _in_maps(inputs), list(range(N_CORES)))
    outs = [res.results[c]["outb"] for c in range(N_CORES)]
    out = np.concatenate(outs, axis=0).astype(np.float32)   # [B, NCI, 128, HW]
    return out.reshape(B, C_IMG, H, W)


def kernel_profiled(inputs, mm_mode=MM_MODE, **kw):
    nc = _get_nc(mm_mode)
    res = run_bass_kernel_spmd(nc, _in_maps(inputs), list(range(N_CORES)), **kw)
    outs = [res.results[c]["outb"] for c in range(N_CORES)]
    out = np.concatenate(outs, axis=0).astype(np.float32)
    return out.reshape(B, C_IMG, H, W), res


# revision 35
# speedup vs baseline: 1708.5612x; 1.2781x over previous
"""Fused cross-attention audio fuser (dense transformer block) on TRN2.

Strategy: batch data-parallel across 8 NeuronCores (B=16 -> 2 batches/core,
no collectives), with a q-major attention pipeline in fp8/bf16:

  kT = Wk.T @ audT (+bk via Act-evac bias)       [C_AUD, K_LEN]  fp8
  v  = audT.T @ Wv (+bv via K=1 matmul)          [K_LEN, C_AUD]  fp8,
       stored per-head with a fused ones column ([K, h, 65])
  qT = Wq.T @ imgT (+bq via Act-evac bias)       [C_AUD, HW]     fp8
  S_h = kT_h.T @ qT_h; expS = exp(SCALE*S)       [K_LEN, HW]     fp8
  attn_q[q, h, d|sum] = expS_h.T @ [v_h | 1]     q-major, the ones column
       of v makes column 64 the softmax denominator for free
  attn_q normalized by per-partition (=per-q) reciprocal (DVE broadcast AP)
  attnT = dma_transpose(attn_q)                  [C_AUD, HW]     bf16
  projT = Wo.T @ attnT + I.T @ (img + bo)        residual+bias fused in psum
  layernorm: stats via (1/C)*ones matmuls (sum at psum partition 0, sum of
       squares at partition 32 of the same bank), Rsqrt row, ones-matmul
       partition broadcast, 3-pass bf16 DVE apply with per-partition
       gamma/beta.

All matmul operands fp8e4 or bf16 (1 cyc/col); exp scale folds the attention
scale; bo is folded into the residual image host-side; bq/bk ride Act
evacuation bias slots; bv rides a K=1 ones matmul (fp8-quantized, exact for
bv=0).
"""

import numpy as np
from contextlib import ExitStack

import concourse.bass as bass
import concourse.mybir as mybir
import concourse.tile as tile
from concourse import bacc
from concourse.bass_utils import run_bass_kernel_spmd
from concourse.masks import make_identity

# Problem constants (hardcoded per spec)
B, C_IMG, H, W = 16, 512, 32, 32
C_AUD, K_LEN, N_HEADS = 512, 256, 8
HD = C_AUD // N_HEADS           # 64
HW = H * W                      # 1024
EPS = 1e-5
SCALE = float(HD) ** -0.5       # 0.125
N_CORES = 8
BPC = B // N_CORES              # 2 batches per core

F32 = mybir.dt.float32
F32R = mybir.dt.float32r
BF16 = mybir.dt.bfloat16
FP8 = mybir.dt.float8e4
P = 128
NCI = C_IMG // P                # 4 img-channel chunks
NCA = C_AUD // P                # 4 audio-channel chunks
NS = K_LEN // P                 # 2 key chunks
NT = 512                        # matmul free tile
NQ = HW // NT                   # 2 q column tiles
QC = HW // P                    # 8 q chunks of 128 (partition dim in q-major)
HB = N_HEADS // 2               # head pairs

MM_MODE = "fp8"                 # informational only
DR = mybir.MatmulPerfMode.DoubleRow

Ident = mybir.ActivationFunctionType.Identity
Copy = mybir.ActivationFunctionType.Copy
Exp = mybir.ActivationFunctionType.Exp
Sqrt = mybir.ActivationFunctionType.Sqrt
Square = mybir.ActivationFunctionType.Square
ADD = mybir.AluOpType.add
SUB = mybir.AluOpType.subtract
MUL = mybir.AluOpType.mult


def _body(ctx: ExitStack, tc: tile.TileContext, dbg=False, repeat=1):
    nc = tc.nc

    audt_d = nc.dram_tensor("audt", [BPC, NCA, P, K_LEN], FP8, kind="ExternalInput").ap()
    img8_d = nc.dram_tensor("img8", [BPC, NCI, P, HW], FP8, kind="ExternalInput").ap()
    imgb_d = nc.dram_tensor("imgb", [BPC, NCI, P, HW], BF16, kind="ExternalInput").ap()
    wq_d = nc.dram_tensor("wq", [NCI, P, C_AUD], FP8, kind="ExternalInput").ap()
    wk_d = nc.dram_tensor("wk", [NCA, P, C_AUD], FP8, kind="ExternalInput").ap()
    wv_d = nc.dram_tensor("wv", [NCA, P, C_AUD], FP8, kind="ExternalInput").ap()
    wo_d = nc.dram_tensor("wo", [NCA, P, C_IMG], FP8, kind="ExternalInput").ap()
    misc_d = nc.dram_tensor("misc", [P, 16], F32, kind="ExternalInput").ap()
    bv_d = nc.dram_tensor("bv", [C_AUD], FP8, kind="ExternalInput").ap()
    out_d = nc.dram_tensor("outb", [BPC, NCI, P, HW], BF16, kind="ExternalOutput").ap()

    if dbg:
        dbg_d = {
            "qT": nc.dram_tensor("dbg_qT", [NCA, P, HW], F32, kind="ExternalOutput").ap(),
            "kT": nc.dram_tensor("dbg_kT", [NCA, P, K_LEN], F32, kind="ExternalOutput").ap(),
            "v8": nc.dram_tensor("dbg_v8", [NS, P, 8 * 65], F32, kind="ExternalOutput").ap(),
            "expS": nc.dram_tensor("dbg_expS", [NS, P, HW], F32, kind="ExternalOutput").ap(),
            "attnq": nc.dram_tensor("dbg_attnq", [QC, P, C_AUD], F32, kind="ExternalOutput").ap(),
            "attnT": nc.dram_tensor("dbg_attnT", [NCA, P, HW], F32, kind="ExternalOutput").ap(),
            "y": nc.dram_tensor("dbg_y", [NCI, P, HW], F32, kind="ExternalOutput").ap(),
        }
        dbg_pool = ctx.enter_context(tc.tile_pool(name="dbgp", bufs=2))

        def dump(dst, src_ap, rows=P):
            t = dbg_pool.tile([rows, src_ap.shape[-1]], F32, tag="dbg")
            nc.scalar.activation(t[:], src_ap, Copy)
            nc.sync.dma_start(out=dst, in_=t[:])

    # ---- pools ----
    cpool = ctx.enter_context(tc.tile_pool(name="consts", bufs=1))
    wpool = ctx.enter_context(tc.tile_pool(name="weights", bufs=1))
    inp_pool = ctx.enter_context(tc.tile_pool(name="inp", bufs=2))
    kv_pool = ctx.enter_context(tc.tile_pool(name="kv", bufs=2))
    qT_pool = ctx.enter_context(tc.tile_pool(name="qT", bufs=2))
    expS_pool = ctx.enter_context(tc.tile_pool(name="expS", bufs=9))
    aq_pool = ctx.enter_context(tc.tile_pool(name="aq", bufs=2))
    y_pool = ctx.enter_context(tc.tile_pool(name="y", bufs=2))
    row_pool = ctx.enter_context(tc.tile_pool(name="rows", bufs=4))
    bc_pool = ctx.enter_context(tc.tile_pool(name="bc", bufs=2))
    chunk_pool = ctx.enter_context(tc.tile_pool(name="chunk", bufs=4))
    mm_ps = ctx.enter_context(tc.tile_pool(name="mm_ps", bufs=2, space="PSUM"))
    sc_ps = ctx.enter_context(tc.tile_pool(name="sc_ps", bufs=2, space="PSUM"))
    at_ps = ctx.enter_context(tc.tile_pool(name="at_ps", bufs=2, space="PSUM"))
    st_ps = mm_ps  # stats psums time-share the mm slots (same tag below)

    # ---- constants ----
    ones_f32 = cpool.tile([P, P], F32, tag="ones_f32")
    nc.vector.memset(ones_f32[:], 1.0)
    ident_f32 = cpool.tile([P, P], F32, tag="ident")
    make_identity(nc, ident_f32[:])
    identb = cpool.tile([P, P], BF16, tag="identb")
    nc.scalar.activation(identb[:], ident_f32[:], Copy)
    ones_row8 = cpool.tile([1, P], FP8, tag="ones_row8")
    nc.scalar.activation(ones_row8[:], ones_f32[0:1, :], Copy)
    ones_rowb = cpool.tile([1, P], BF16, tag="ones_rowb")
    nc.scalar.activation(ones_rowb[:], ones_f32[0:1, :], Copy)
    oneC_col = cpool.tile([P, 1], BF16, tag="oneC")   # 1/C_IMG for LN stats
    invc = cpool.tile([P, 1], F32, tag="invc")
    nc.vector.memset(invc[:], 1.0 / C_IMG)
    nc.scalar.activation(oneC_col[:], invc[:], Copy)
    eps_col = cpool.tile([1, 1], F32, tag="eps")
    nc.vector.memset(eps_col[:], EPS)

    wq_sb = wpool.tile([P, NCI, C_AUD], FP8, tag="wq")
    wk_sb = wpool.tile([P, NCA, C_AUD], FP8, tag="wk")
    wv_sb = wpool.tile([P, NCA, C_AUD], FP8, tag="wv")
    wo_sb = wpool.tile([P, NCA, C_IMG], FP8, tag="wo")
    misc_sb = cpool.tile([P, 16], F32, tag="misc")
    bq_col = misc_sb[:, 0:4]
    bk_col = misc_sb[:, 4:8]
    gam_col = misc_sb[:, 8:12]
    bet_col = misc_sb[:, 12:16]
    bv_row = cpool.tile([1, C_AUD], FP8, tag="bv")

    for rep in range(repeat):
        # ---- input + weight DMAs (ordered to feed the first consumers) ----
        aud_tiles, img8_tiles, imgb_tiles = [], [], []
        for b in range(BPC):
            aud_tiles.append(inp_pool.tile([P, NCA, K_LEN], FP8, tag="aud", name=f"aud{b}"))
            img8_tiles.append(inp_pool.tile([P, NCI, HW], FP8, tag="img8", name=f"img8_{b}"))
            imgb_tiles.append(inp_pool.tile([P, NCI, HW], BF16, tag="imgb", name=f"imgb{b}"))
        nc.sync.dma_start(out=wk_sb[:], in_=wk_d.rearrange("c p m -> p c m"))
        for b in range(BPC):
            nc.sync.dma_start(out=aud_tiles[b][:],
                              in_=audt_d[b].rearrange("c p k -> p c k"))
        nc.sync.dma_start(out=misc_sb[:], in_=misc_d)
        nc.sync.dma_start(out=wv_sb[:], in_=wv_d.rearrange("c p m -> p c m"))
        nc.sync.dma_start(out=bv_row[:], in_=bv_d[:])
        nc.sync.dma_start(out=wq_sb[:], in_=wq_d.rearrange("c p m -> p c m"))
        for b in range(BPC):
            nc.sync.dma_start(out=img8_tiles[b][:],
                              in_=img8_d[b].rearrange("c p q -> p c q"))
        nc.sync.dma_start(out=wo_sb[:], in_=wo_d.rearrange("c p m -> p c m"))
        for b in range(BPC):
            nc.sync.dma_start(out=imgb_tiles[b][:],
                              in_=imgb_d[b].rearrange("c p q -> p c q"))

        # ---- per-batch state ----
        kT8 = [None] * BPC      # [128, NCA, K_LEN] fp8, head-pair-major
        v8 = [None] * BPC       # [128, NS, 8*65] fp8 (65th col of each head = 1)
        qT8 = [None] * BPC      # [128, NCA, HW] fp8
        expS8 = [None] * BPC    # per head: [128, NS, HW] fp8
        aqbf = [None] * BPC     # [128, QC, C_AUD] bf16 (q-major attn)
        attnT = [None] * BPC    # [128, NCA, HW] bf16
        ybf = [None] * BPC      # [128, NCI, HW] bf16
        nmu_bf = [None] * BPC
        rstd_bf = [None] * BPC

        def ph_kT(b):
            kT8[b] = kv_pool.tile([P, NCA, K_LEN], FP8, tag="kT", name=f"kT{b}")
            for m in range(NCA):
                ps = mm_ps.tile([P, K_LEN], F32, tag="mm")
                for ci in range(NCA):
                    nc.tensor.matmul(ps[:], wk_sb[:, ci, m * P:(m + 1) * P],
                                     aud_tiles[b][:, ci, :],
                                     start=(ci == 0), stop=(ci == NCA - 1))
                with nc.allow_low_precision(reason="fp8 kT evac"):
                    nc.vector.tensor_scalar_add(kT8[b][:, m, :], ps[:],
                                                bk_col[:, m:m + 1])
                if dbg and b == 0:
                    dump(dbg_d["kT"][m], kT8[b][:, m, :])

        def ph_v(b):
            v8[b] = kv_pool.tile([P, NS, 8 * 65], FP8, tag="v8", name=f"v8_{b}")
            # ones columns (col 64 of each head block)
            nc.scalar.activation(
                v8[b][:].rearrange("p k (h x) -> p k h x", h=8)[:, :, :, 64],
                ones_f32[:, 0:16].rearrange("p (k h) -> p k h", k=NS), Copy)
            for kt in range(NS):
                ps = mm_ps.tile([P, C_AUD], F32, tag="mm")
                for ci in range(NCA):
                    nc.tensor.matmul(ps[:], aud_tiles[b][:, ci, kt * P:(kt + 1) * P],
                                     wv_sb[:, ci, :],
                                     start=(ci == 0), stop=False)
                nc.tensor.matmul(ps[:], ones_row8[:], bv_row[:],
                                 start=False, stop=True)
                nc.vector.tensor_copy(
                    v8[b][:, kt, :].rearrange("p (h x) -> p h x", h=8)[:, :, 0:64],
                    ps[:].rearrange("p (h x) -> p h x", h=8))
                if dbg and b == 0:
                    dump(dbg_d["v8"][kt], v8[b][:, kt, :])

        def ph_qT(b, ms=range(NCA)):
            if qT8[b] is None:
                qT8[b] = qT_pool.tile([P, NCA, HW], FP8, tag="qT", name=f"qT{b}")
            for m in ms:
                for n in range(NQ):
                    ps = mm_ps.tile([P, NT], F32, tag="mm")
                    for ci in range(NCI):
                        nc.tensor.matmul(ps[:], wq_sb[:, ci, m * P:(m + 1) * P],
                                         img8_tiles[b][:, ci, n * NT:(n + 1) * NT],
                                         start=(ci == 0), stop=(ci == NCI - 1))
                    with nc.allow_low_precision(reason="fp8 qT evac"):
                        nc.vector.tensor_scalar_add(
                            qT8[b][:, m, n * NT:(n + 1) * NT], ps[:],
                            bq_col[:, m:m + 1])
                if dbg and b == 0:
                    dump(dbg_d["qT"][m], qT8[b][:, m, :])

        def ph_se(b, pair):
            if expS8[b] is None:
                expS8[b] = [None] * N_HEADS
            for h in (2 * pair, 2 * pair + 1):
                ht, hr = h // 2, (h % 2) * HD
                et = expS_pool.tile([P, NS, HW], FP8, tag="expS", name=f"expS{b}_{h}")
                for kt in range(NS):
                    ps = sc_ps.tile([P, HW], F32, tag="sc")
                    for n in range(NQ):
                        nc.tensor.matmul(
                            ps[:, n * NT:(n + 1) * NT],
                            kT8[b][hr:hr + HD, ht, kt * P:(kt + 1) * P],
                            qT8[b][hr:hr + HD, ht, n * NT:(n + 1) * NT],
                            start=True, stop=True)
                    nc.scalar.activation(et[:, kt, :], ps[:], Exp, scale=SCALE)
                    if dbg and b == 0 and h == 0:
                        dump(dbg_d["expS"][kt], et[:, kt, :])
                expS8[b][h] = et

        def ph_attn(b, half, qcb):
            # one rung: attn matmuls for 4 heads x 2 q-chunks, then normalize
            if aqbf[b] is None:
                aqbf[b] = aq_pool.tile([P, QC, C_AUD], BF16, tag="aq", name=f"aq{b}")
                attnT[b] = aq_pool.tile([P, NCA, HW], BF16, tag="attnT",
                                        name=f"aT{b}")
            for qc in (2 * qcb, 2 * qcb + 1):
                pa = at_ps.tile([P, 4, 65], F32, tag="at", name=f"pa{b}_{half}_{qc}")
                for hh in range(4):
                    h = 4 * half + hh
                    for kt in range(NS):
                        nc.tensor.matmul(
                            pa[:, hh, :],
                            expS8[b][h][:, kt, qc * P:(qc + 1) * P],
                            v8[b][:, kt, h * 65:(h + 1) * 65],
                            start=(kt == 0), stop=(kt == NS - 1))
                rq = chunk_pool.tile([P, 4], F32, tag="rq")
                nc.vector.reciprocal(rq[:], pa[:, :, 64])
                nc.vector.tensor_tensor(
                    aqbf[b][:, qc, half * 256:(half + 1) * 256]
                    .rearrange("p (h x) -> p h x", h=4),
                    pa[:, :, 0:64],
                    rq[:].unsqueeze(-1).broadcast_to([P, 4, 64]), MUL)
                if dbg and b == 0 and half == 1:
                    dump(dbg_d["attnq"][qc], aqbf[b][:, qc, :])

        def ph_trans(b, qcs=range(QC)):
            for qc in qcs:
                nc.sync.dma_start_transpose(
                    attnT[b][:, :, qc * P:(qc + 1) * P], aqbf[b][:, qc, :])
            if dbg and b == 0 and list(qcs)[-1] == QC - 1:
                for ci in range(NCA):
                    dump(dbg_d["attnT"][ci], attnT[b][:, ci, :])

        def ph_proj(b, ms=range(NCI), ns=range(NQ)):
            if ybf[b] is None:
                ybf[b] = y_pool.tile([P, NCI, HW], BF16, tag="y", name=f"y{b}")
            for m in ms:
                for n in ns:
                    ps = mm_ps.tile([P, NT], F32, tag="mm")
                    for ca in range(NCA):
                        nc.tensor.matmul(ps[:], wo_sb[:, ca, m * P:(m + 1) * P],
                                         attnT[b][:, ca, n * NT:(n + 1) * NT],
                                         start=(ca == 0), stop=False)
                    nc.tensor.matmul(ps[:], identb[:],
                                     imgb_tiles[b][:, m, n * NT:(n + 1) * NT],
                                     start=False, stop=True, skip_group_check=True)
                    nc.scalar.activation(ybf[b][:, m, n * NT:(n + 1) * NT], ps[:],
                                         Copy)
                if dbg and b == 0 and list(ns)[-1] == NQ - 1:
                    dump(dbg_d["y"][m], ybf[b][:, m, :])

        def ph_stats(b):
            stp = [st_ps.tile([64, NT], F32, tag="mm", name=f"st{b}_{n}")
                   for n in range(NQ)]
            for ci in range(NCI):
                for n in range(NQ):
                    nc.tensor.matmul(stp[n][0:1, :], oneC_col[:],
                                     ybf[b][:, ci, n * NT:(n + 1) * NT],
                                     start=(ci == 0), stop=(ci == NCI - 1))
            for ci in range(NCI):
                ysq = chunk_pool.tile([P, HW], BF16, tag="ysq")
                nc.vector.tensor_tensor(ysq[:], ybf[b][:, ci, :], ybf[b][:, ci, :], MUL)
                for n in range(NQ):
                    nc.tensor.matmul(stp[n][32:33, :], oneC_col[:],
                                     ysq[:, n * NT:(n + 1) * NT],
                                     start=(ci == 0), stop=(ci == NCI - 1))
            # rows: mean at partition 0, E[y^2] at partition 32
            nmu_bf[b] = row_pool.tile([1, HW], BF16, tag="row", name=f"nmu{b}")
            rstd_bf[b] = row_pool.tile([1, HW], BF16, tag="row", name=f"rstd{b}")
            for n in range(NQ):
                sl = slice(n * NT, (n + 1) * NT)
                mu2 = row_pool.tile([1, NT], F32, tag="mu2")
                nc.scalar.activation(mu2[:], stp[n][0:1, :], Square)
                var = row_pool.tile([1, NT], F32, tag="var")
                nc.vector.tensor_tensor(var[:], stp[n][32:33, :], mu2[:], SUB)
                std = row_pool.tile([1, NT], F32, tag="std")
                nc.scalar.activation(std[:], var[:], Sqrt, bias=eps_col[:])
                with nc.allow_low_precision(reason="bf16 rstd row; |err|<2^-9 ok"):
                    nc.vector.reciprocal(rstd_bf[b][:, sl], std[:])
                nc.vector.tensor_scalar_mul(nmu_bf[b][:, sl], stp[n][0:1, :], -1.0)
            # partition broadcasts via K=1 ones matmuls
            nbc = bc_pool.tile([P, HW], BF16, tag="nbc", name=f"nbc{b}")
            rbc = bc_pool.tile([P, HW], BF16, tag="rbc", name=f"rbc{b}")
            for n in range(NQ):
                sl = slice(n * NT, (n + 1) * NT)
                b1 = mm_ps.tile([P, NT], F32, tag="mm")
                nc.tensor.matmul(b1[:], ones_rowb[:], nmu_bf[b][:, sl],
                                 start=True, stop=True)
                nc.scalar.activation(nbc[:, sl], b1[:], Copy)
                b2 = mm_ps.tile([P, NT], F32, tag="mm")
                nc.tensor.matmul(b2[:], ones_rowb[:], rstd_bf[b][:, sl],
                                 start=True, stop=True)
                nc.scalar.activation(rbc[:, sl], b2[:], Copy)
            nmu_bf[b], rstd_bf[b] = nbc, rbc

        def ph_apply(b):
            obf = y_pool.tile([P, NCI, HW], BF16, tag="obf", name=f"obf{b}")
            for n in range(NQ):
                sl = slice(n * NT, (n + 1) * NT)
                for ci in range(NCI):
                    t1 = chunk_pool.tile([P, NT], BF16, tag="t1")
                    nc.vector.tensor_tensor(t1[:], ybf[b][:, ci, sl],
                                            nmu_bf[b][:, sl], ADD)
                    t2 = chunk_pool.tile([P, NT], BF16, tag="t2")
                    nc.vector.tensor_tensor(t2[:], t1[:], rstd_bf[b][:, sl], MUL)
                    nc.scalar.activation(obf[:, ci, sl], t2[:], Ident,
                                         scale=gam_col[:, ci:ci + 1],
                                         bias=bet_col[:, ci:ci + 1])
                nc.sync.dma_start(
                    out=out_d[b].rearrange("c p q -> p c q")[:, :, sl],
                    in_=obf[:, :, sl])

        # ---- phase schedule (emission order == per-engine execution order) ----
        ph_kT(0)
        ph_v(0)
        ph_qT(0)
        # ladder: rungs of (4 heads x 2 q-chunks) interleaved with score/exp
        ph_se(0, 0)
        ph_se(0, 1)
        ph_kT(1)
        ph_attn(0, 0, 0)
        ph_attn(0, 0, 1)
        ph_se(0, 2)
        ph_v(1)
        ph_attn(0, 0, 2)
        ph_attn(0, 0, 3)
        ph_se(0, 3)
        ph_qT(1, range(0, 2))
        ph_attn(0, 1, 0)
        ph_attn(0, 1, 1)
        ph_qT(1, range(2, 4))
        ph_attn(0, 1, 2)
        ph_trans(0, range(0, 4))
        ph_attn(0, 1, 3)
        ph_trans(0, range(4, 8))
        ph_se(1, 0)
        ph_se(1, 1)
        ph_proj(0, range(0, 2))
        ph_attn(1, 0, 0)
        ph_attn(1, 0, 1)
        ph_se(1, 2)
        ph_proj(0, range(2, 4))
        ph_attn(1, 0, 2)
        ph_attn(1, 0, 3)
        ph_se(1, 3)
        ph_stats(0)
        ph_attn(1, 1, 0)
        ph_attn(1, 1, 1)
        ph_trans(1, range(0, 4))
        ph_attn(1, 1, 2)
        ph_proj(1, ns=range(0, 1))
        ph_attn(1, 1, 3)
        ph_trans(1, range(4, 8))
        ph_proj(1, ns=range(1, 2))
        ph_stats(1)
        ph_apply(0)
        ph_apply(1)


def build(mm_mode=MM_MODE, dbg=False, repeat=1):
    nc = bacc.Bacc("TRN2", target_bir_lowering=False, debug=False)
    with tile.TileContext(nc) as tc, ExitStack() as ctx:
        _body(ctx, tc, dbg=dbg, repeat=repeat)
    nc.compile()
    return nc


def build_debug(mm_mode=MM_MODE):
    return build(mm_mode, dbg=True)


_NC_CACHE = {}


def _get_nc(mm_mode=MM_MODE):
    if mm_mode not in _NC_CACHE:
        _NC_CACHE[mm_mode] = build(mm_mode)
    return _NC_CACHE[mm_mode]


NP8 = mybir.dt.np(FP8)
NPBF = mybir.dt.np(BF16)


def _in_maps(inputs):
    img = np.asarray(inputs["img_feat"], np.float32).reshape(B, C_IMG, HW)
    aud = np.asarray(inputs["audio_feat"], np.float32)
    bo = np.asarray(inputs["bo"], np.float32)

    # audT: [B, K, C] -> [B, C, K] -> chunked [B, NCA, 128, K]
    audt = np.ascontiguousarray(aud.transpose(0, 2, 1)).reshape(B, NCA, P, K_LEN)
    img8 = img.reshape(B, NCI, P, HW)
    imgb = (img + bo[None, :, None]).reshape(B, NCI, P, HW)

    def wprep(w):
        return np.ascontiguousarray(np.asarray(w, np.float32)
                                    .reshape(NCA, P, -1)).astype(NP8)

    misc = np.stack([
        np.asarray(inputs["bq"], np.float32).reshape(4, P),
        np.asarray(inputs["bk"], np.float32).reshape(4, P),
        np.asarray(inputs["gamma"], np.float32).reshape(4, P),
        np.asarray(inputs["beta"], np.float32).reshape(4, P),
    ]).reshape(16, P).T.copy()          # [128, 16] = bq|bk|gamma|beta columns
    shared = {
        "wq": wprep(inputs["Wq"]),
        "wk": wprep(inputs["Wk"]),
        "wv": wprep(inputs["Wv"]),
        "wo": wprep(inputs["Wo"]),
        "misc": misc,
        "bv": np.asarray(inputs["bv"], np.float32).astype(NP8),
    }
    maps = []
    for c in range(N_CORES):
        sl = slice(c * BPC, (c + 1) * BPC)
        maps.append({
            "audt": audt[sl].astype(NP8),
            "img8": img8[sl].astype(NP8),
            "imgb": imgb[sl].astype(NPBF),
            **shared,
        })
    return maps


def kernel(**inputs) -> np.ndarray:
    nc = _get_nc()
    res = run_bass_kernel_spmd(nc, _in_maps(inputs), list(range(N_CORES)))
    outs = [res.results[c]["outb"] for c in range(N_CORES)]
    out = np.concatenate(outs, axis=0).astype(np.float32)   # [B, NCI, 128, HW]
    return out.reshape(B, C_IMG, H, W)


def kernel_profiled(inputs, mm_mode=MM_MODE, **kw):
    nc = _get_nc(mm_mode)
    res = run_bass_kernel_spmd(nc, _in_maps(inputs), list(range(N_CORES)), **kw)
    outs = [res.results[c]["outb"] for c in range(N_CORES)]
    out = np.concatenate(outs, axis=0).astype(np.float32)
    return out.reshape(B, C_IMG, H, W), res
